# revision 3
# baseline (speedup 1.0000x reference)
"""Trainium2 Bass kernel for nn_NodeModel (GNN message passing).

reference:
    agg = segment_sum(edge_attr, edge_index[0], num_segments=100000)   # [N, 64]
    h = concat([x, agg, u[v_indices]], axis=1)                         # [N, 256]
    out = relu(h @ W1 + b1) @ W2 + b2                                  # [N, 128]

Strategy (8 NeuronCores, SPMD, no collectives):
  - Nodes are assigned to (core, window-of-32) slots by a degree-balanced
    snake deal: nodes sorted by degree are dealt round-robin (alternating
    direction) across all 3200 (core, window) bins, so every bin holds ~1/3200
    of all edges and almost every window needs exactly 4 zero-padded 128-edge
    tiles (the padding-free minimum is 3.9). Output is un-permuted on host.
  - edge_attr is sent as bf16 (hi half only; ~0.1% rel err, well under the
    2e-2 gate) => half the dominant HBM traffic. Everything else is bf16 too
    except biases; the output returns as bf16 and is widened on host.
  - DMA granularity: one ea/x/ug/out DMA per 8-block supergroup (1024 nodes).
    The HWDGE descriptor-generation path costs ~630ns per DMA instruction,
    serialized, so few fat DMAs matter as much as few bytes.
  - segment_sum on device: one-hot P[e, (m, t)] = (idx[e, t] == m) built for
    ALL tiles of a 128-node block in ONE DVE tensor_tensor. Layout [128, 32,
    T] with a materialized repeated-iota constant keeps every operand's last
    dim stride-1 so the DVE 2x 16-bit mode applies. Then TensorE matmuls
    aggT += ea.T @ P[:, :, t] per 32-node window, accumulated in PSUM; one
    PSUM tile and one ACT copy per 4-block MLP group.
  - MLP feature-major in bf16. agg (64 rows) and u-gather (64 rows) share one
    128-partition tile so h@W1 is 2 matmuls per output half instead of 3.
    ReLU is split between ACT (mh=0) and DVE (mh=1) to balance engines.
"""

import sys

sys.path.insert(0, "/opt/trn_rl_repo")

import numpy as np
import ml_dtypes

import concourse.bass as bass
import concourse.mybir as mybir
from concourse import bacc, tile
from concourse.bass_utils import run_bass_kernel_spmd

bf16 = ml_dtypes.bfloat16

D_X, D_E, D_U = 128, 64, 64
D_HID, D_OUT = 256, 128
NB = 128   # nodes per block
WSZ = 32   # nodes per one-hot window
WIN = NB // WSZ
SGB = 8    # blocks per DMA supergroup

FULL_CFG = dict(
    n_cores=8, n_nodes=100000, blocks=100, group=4
)  # 12800 node slots/core

_cache = {}


def _build_nc(Tb, blocks, npad, group, n_cores=8, reps=1, opts=None):
    """Build the SPMD Bass program. Tb = per-window edge tile counts
    (shared across cores; windows are 32 nodes, WIN windows per block).

    reps > 1 wraps the computation in a hardware For_i loop — used only
    for timing (per-iter time = delta(wall)/delta(reps), cancelling the
    host dispatch overhead)."""
    opts = dict(opts or {})
    ea_bufs = opts.get("ea_bufs", 3)
    p_bufs = opts.get("p_bufs", 6)
    Tb = list(Tb)
    offs = [0]
    for t in Tb:
        offs.append(offs[-1] + t)
    TT = offs[-1]
    max_blk_tiles = max(
        sum(Tb[b * WIN : (b + 1) * WIN]) for b in range(blocks)
    )
    sgs = [
        (s, min(s + SGB, blocks)) for s in range(0, blocks, SGB)
    ]
    max_sg_tiles = max(
        sum(Tb[s * WIN : e * WIN]) for s, e in sgs
    )
    nc = bacc.Bacc(
        "TRN2", target_bir_lowering=False, debug=False, num_devices=n_cores
    )
    f32, b16 = mybir.dt.float32, mybir.dt.bfloat16

    GW = group * NB       # nodes per MLP group
    SGW = SGB * NB        # nodes per supergroup
    gpsg = SGB // group   # MLP groups per supergroup

    # partition-major layouts; [K, mh, M] for weights
    ea_in = nc.declare_dram_parameter("ea", [128, TT * 64], b16, isOutput=False)
    idx_in = nc.declare_dram_parameter("idx", [128, TT], b16, isOutput=False)
    iota_in = nc.declare_dram_parameter(
        "iota", [128, WSZ * max_blk_tiles], b16, isOutput=False
    )
    xT_in = nc.declare_dram_parameter("xT", [128, npad], b16, isOutput=False)
    ugT_in = nc.declare_dram_parameter("ugT", [64, npad], b16, isOutput=False)
    w1x_in = nc.declare_dram_parameter("w1x", [128, 2, 128], b16, isOutput=False)
    w1au_in = nc.declare_dram_parameter("w1au", [128, 2, 128], b16, isOutput=False)
    w2_in = nc.declare_dram_parameter("w2", [128, 2, 128], b16, isOutput=False)
    b1_in = nc.declare_dram_parameter("b1", [128, 2], f32, isOutput=False)
    b2_in = nc.declare_dram_parameter("b2", [128, 1], f32, isOutput=False)
    outT = nc.declare_dram_parameter("outT", [128, npad], b16, isOutput=True)

    with tile.TileContext(nc) as tc:
        with (
            tc.tile_pool(name="const", bufs=1) as cpool,
            tc.tile_pool(name="x", bufs=3) as xpool,
            tc.tile_pool(name="ea", bufs=ea_bufs) as eapool,
            tc.tile_pool(name="p", bufs=p_bufs) as ppool,
            tc.tile_pool(name="aug", bufs=3) as augpool,
            tc.tile_pool(name="h1", bufs=4) as h1pool,
            tc.tile_pool(name="outs", bufs=3) as opool,
            tc.tile_pool(name="ps_agg", bufs=3, space="PSUM") as agg_ps_pool,
            tc.tile_pool(name="ps_o1", bufs=2, space="PSUM") as o1_ps_pool,
            tc.tile_pool(name="ps_o2", bufs=2, space="PSUM") as o2_ps_pool,
        ):
          def _emit_body():
              # ---- constants ----
              idx_t = cpool.tile([128, TT], b16, tag="idx")
              nc.sync.dma_start(idx_t[:], idx_in[:])
              iota_t = cpool.tile([128, WSZ, max_blk_tiles], b16, tag="iota")
              nc.sync.dma_start(iota_t[:], iota_in[:])
              w1x_t = cpool.tile([128, 2, 128], b16, tag="w1x")
              nc.sync.dma_start(w1x_t[:], w1x_in[:])
              w1au_t = cpool.tile([128, 2, 128], b16, tag="w1au")
              nc.sync.dma_start(w1au_t[:], w1au_in[:])
              w2_t = cpool.tile([128, 2, 128], b16, tag="w2")
              nc.sync.dma_start(w2_t[:], w2_in[:])
              b1_t = cpool.tile([128, 2], f32, tag="b1")
              nc.sync.dma_start(b1_t[:], b1_in[:])
              b2_t = cpool.tile([128, 1], f32, tag="b2")
              nc.sync.dma_start(b2_t[:], b2_in[:])

              for sg, (bs, be) in enumerate(sgs):
                  nsg = (be - bs) * NB
                  s = bs * NB
                  o_sg = offs[bs * WIN]
                  Tsg = offs[be * WIN] - o_sg
                  # ---- supergroup DMAs ----
                  ea_t = eapool.tile(
                      [128, max_sg_tiles * 64], b16, tag="ea", name=f"ea{sg}"
                  )
                  nc.sync.dma_start(
                      ea_t[:, : Tsg * 64],
                      ea_in[:, o_sg * 64 : (o_sg + Tsg) * 64],
                  )
                  x_t = xpool.tile([128, SGW], b16, tag="x")
                  nc.sync.dma_start(x_t[:, :nsg], xT_in[:, s : s + nsg])
                  aug = augpool.tile([128, SGW], b16, tag="aug")
                  nc.scalar.dma_start(
                      aug[64:128, :nsg], ugT_in[:, s : s + nsg]
                  )
                  out_t = opool.tile([128, SGW], b16, tag="outs")

                  for g in range(gpsg):
                      gb = bs + g * group
                      if gb >= be:
                          break
                      gw = (min(gb + group, be) - gb) * NB
                      go = g * group * NB    # offset within supergroup
                      agg_ps = agg_ps_pool.tile([64, GW], f32, tag="agg")
                      for bi in range(group):
                          b = gb + bi
                          if b >= be:
                              break
                          Tws = Tb[b * WIN : (b + 1) * WIN]
                          o_b = offs[b * WIN] - o_sg   # tile offset in ea_t
                          Tblk = sum(Tws)
                          # one-hot for all tiles of the block in ONE DVE op
                          p_t = ppool.tile(
                              [128, WSZ, max_blk_tiles], b16, tag="p"
                          )
                          nc.vector.tensor_tensor(
                              out=p_t[:, :, 0:Tblk],
                              in0=idx_t[:, o_sg + o_b : o_sg + o_b + Tblk]
                              .unsqueeze(1)
                              .broadcast_to([128, WSZ, Tblk]),
                              in1=iota_t[:, :, 0:Tblk],
                              op=mybir.AluOpType.is_equal,
                          )
                          # segment-sum via matmul per tile, acc per window
                          ti = 0
                          for w in range(WIN):
                              co = bi * NB + w * WSZ
                              for t in range(Tws[w]):
                                  nc.tensor.matmul(
                                      agg_ps[:, co : co + WSZ],
                                      ea_t[:, (o_b + ti) * 64
                                           : (o_b + ti + 1) * 64],
                                      p_t[:, :, ti],
                                      start=(t == 0),
                                      stop=(t == Tws[w] - 1),
                                  )
                                  ti += 1
                      # stage aggT into the combined [agg; ug] tile
                      nc.scalar.activation(
                          out=aug[0:64, go : go + gw],
                          in_=agg_ps[:, :gw],
                          func=mybir.ActivationFunctionType.Copy,
                      )
                      # ---- MLP for this group ----
                      h1_list = []
                      for mh in range(2):
                          o1 = o1_ps_pool.tile([128, GW], f32, tag="o1")
                          nc.tensor.matmul(
                              o1[:, :gw], w1x_t[:, mh, :],
                              x_t[:, go : go + gw],
                              start=True, stop=False,
                          )
                          nc.tensor.matmul(
                              o1[:, :gw], w1au_t[:, mh, :],
                              aug[:, go : go + gw],
                              start=False, stop=True,
                          )
                          h1 = h1pool.tile([128, GW], b16, tag="h1")
                          if mh == 0:
                              nc.scalar.activation(
                                  out=h1[:, :gw], in_=o1[:, :gw],
                                  func=mybir.ActivationFunctionType.Relu,
                                  bias=b1_t[:, 0:1],
                              )
                          else:
                              nc.vector.tensor_scalar(
                                  out=h1[:, :gw], in0=o1[:, :gw],
                                  scalar1=b1_t[:, 1:2], scalar2=0.0,
                                  op0=mybir.AluOpType.add,
                                  op1=mybir.AluOpType.max,
                              )
                          h1_list.append(h1)
                      o2 = o2_ps_pool.tile([128, GW], f32, tag="o2")
                      for kh in range(2):
                          nc.tensor.matmul(
                              o2[:, :gw], w2_t[:, kh, :],
                              h1_list[kh][:, :gw],
                              start=(kh == 0), stop=(kh == 1),
                          )
                      nc.scalar.activation(
                          out=out_t[:, go : go + gw], in_=o2[:, :gw],
                          func=mybir.ActivationFunctionType.Identity,
                          bias=b2_t[:],
                      )
                  nc.scalar.dma_start(
                      outT[:, s : s + nsg], out_t[:, :nsg]
                  )

          if reps == 1:
              _emit_body()
          else:
              with tc.For_i(0, reps, 1):
                  _emit_body()

    nc.compile()
    return nc


def _pack_inputs(x, edge_index, edge_attr, u, v_indices, W1, b1, W2, b2, cfg):
    """Host-side sharding: degree-balanced node permutation + edge packing."""
    n_cores, blocks = cfg["n_cores"], cfg["blocks"]
    n_nodes = cfg["n_nodes"]
    npad = blocks * NB
    nwin = npad // WSZ           # windows per core
    nbins = n_cores * nwin       # (core, window) bins
    nslots = nbins * WSZ
    row = np.asarray(edge_index[0], dtype=np.int64)
    ea = np.asarray(edge_attr, dtype=np.float32)
    x = np.asarray(x, dtype=np.float32)
    u = np.asarray(u, dtype=np.float32)
    v_indices = np.asarray(v_indices, dtype=np.int64)
    W1 = np.asarray(W1, dtype=np.float32)
    W2 = np.asarray(W2, dtype=np.float32)
    b1 = np.asarray(b1, dtype=np.float32)
    b2 = np.asarray(b2, dtype=np.float32)
    d_e = ea.shape[1]

    # ---- snake-deal nodes (sorted by degree desc) across bins ----
    deg = np.bincount(row, minlength=n_nodes)
    order = np.argsort(-deg, kind="stable")          # high degree first
    node_core = np.empty(n_nodes, np.int32)
    node_win = np.empty(n_nodes, np.int32)
    node_off = np.empty(n_nodes, np.int32)
    pos = np.arange(nslots)
    rounds, cols = pos // nbins, pos % nbins
    bins = np.where(rounds % 2 == 0, cols, nbins - 1 - cols)
    rb, bb = rounds[:n_nodes], bins[:n_nodes]
    node_core[order] = (bb // nwin).astype(np.int32)
    node_win[order] = (bb % nwin).astype(np.int32)
    node_off[order] = rb.astype(np.int32)
    node_plocal = node_win * WSZ + node_off          # slot within core

    # ---- edge buckets ----
    ec = node_core[row]
    ew = node_win[row]
    em = node_off[row]
    key = ec.astype(np.int64) * nwin + ew
    cnt = np.bincount(key, minlength=nbins).reshape(n_cores, nwin)
    Tb = np.maximum(1, (cnt.max(axis=0) + 127) // 128).astype(int)  # [nwin]
    offs = np.concatenate([[0], np.cumsum(Tb)])
    TT = int(offs[-1])

    order_e = np.argsort(key, kind="stable")
    key_s = key[order_e]
    cnt_flat = np.bincount(key_s, minlength=nbins)
    starts_flat = np.concatenate([[0], np.cumsum(cnt_flat)])[:-1]
    rank = np.arange(len(key_s)) - starts_flat[key_s]
    ew_s = ew[order_e]
    slot = offs[ew_s] * 128 + rank                   # within-core slot
    ec_s = ec[order_e]
    em_s = em[order_e].astype(np.float32)
    ea_hi = ea[order_e].astype(bf16)

    ea_pack = np.empty((n_cores, 128, TT * 64), dtype=bf16)
    idx_pack = np.empty((n_cores, 128, TT), dtype=bf16)
    for c in range(n_cores):
        m = ec_s == c
        coreslots = np.zeros((TT * 128, d_e), dtype=bf16)
        coreslots[slot[m]] = ea_hi[m]
        ea_pack[c] = (
            coreslots.reshape(TT, 128, d_e).transpose(1, 0, 2).reshape(128, -1)
        )
        ivals = np.zeros(TT * 128, dtype=np.float32)
        ivals[slot[m]] = em_s[m]
        idx_pack[c] = ivals.reshape(TT, 128).T.astype(bf16)

    max_blk_tiles = max(
        sum(Tb[b * WIN : (b + 1) * WIN]) for b in range(blocks)
    )
    # iota2[p, m, t] = m  (materialized so the DVE one-hot reads stride-1)
    iota = np.broadcast_to(
        np.arange(WSZ, dtype=np.float32)[None, :, None],
        (128, WSZ, max_blk_tiles),
    ).reshape(128, -1).astype(bf16)
    uT = u.T  # [d_u, n_graphs]

    w1x = np.ascontiguousarray(W1[:D_X].reshape(D_X, 2, 128)).astype(bf16)
    w1au = np.ascontiguousarray(W1[D_X:].reshape(128, 2, 128)).astype(bf16)
    w2 = np.ascontiguousarray(
        W2.reshape(2, 128, D_OUT).transpose(1, 0, 2)
    ).astype(bf16)
    b1p = np.ascontiguousarray(b1.reshape(2, 128).T)
    b2p = np.ascontiguousarray(b2.reshape(128, 1))

    in_maps = []
    for c in range(n_cores):
        sel = node_core == c
        pl = node_plocal[sel]
        xT = np.zeros((D_X, npad), dtype=bf16)
        xT[:, pl] = x[sel].T.astype(bf16)
        ugT = np.zeros((D_U, npad), dtype=bf16)
        ugT[:, pl] = uT[:, v_indices[sel]].astype(bf16)
        in_maps.append({
            "ea": ea_pack[c],
            "idx": idx_pack[c],
            "iota": iota,
            "xT": xT,
            "ugT": ugT,
            "w1x": w1x,
            "w1au": w1au,
            "w2": w2,
            "b1": b1p,
            "b2": b2p,
        })
    unperm = (node_core, node_plocal)
    return in_maps, tuple(int(t) for t in Tb), unperm


def _unpack_output(res_per_core, unperm, cfg):
    node_core, node_plocal = unperm
    n_nodes = cfg["n_nodes"]
    out = np.empty((n_nodes, D_OUT), dtype=np.float32)
    for c in range(cfg["n_cores"]):
        sel = node_core == c
        out[sel] = np.asarray(res_per_core[c]).astype(np.float32).T[
            node_plocal[sel]
        ]
    return out


def _run(inputs, cfg, reps=1):
    in_maps, T, unperm = _pack_inputs(
        inputs["x"], inputs["edge_index"], inputs["edge_attr"], inputs["u"],
        inputs["v_indices"], inputs["W1"], inputs["b1"], inputs["W2"],
        inputs["b2"], cfg,
    )
    key = (T, cfg["blocks"], cfg["group"], reps)
    if key not in _cache:
        _cache[key] = _build_nc(
            T, cfg["blocks"], cfg["blocks"] * NB, cfg["group"],
            n_cores=cfg["n_cores"], reps=reps,
        )
    nc = _cache[key]
    res = run_bass_kernel_spmd(nc, in_maps, list(range(cfg["n_cores"])))
    return _unpack_output(
        [res.results[c]["outT"] for c in range(cfg["n_cores"])], unperm, cfg
    )


def kernel(x, edge_index, edge_attr, u, v_indices, W1, b1, W2, b2):
    inputs = dict(x=x, edge_index=edge_index, edge_attr=edge_attr, u=u,
                  v_indices=v_indices, W1=W1, b1=b1, W2=W2, b2=b2)
    return _run(inputs, FULL_CFG)


# revision 5
# speedup vs baseline: 1.3448x; 1.3448x over previous
"""Trainium2 Bass kernel for nn_NodeModel (GNN message passing).

reference:
    agg = segment_sum(edge_attr, edge_index[0], num_segments=100000)   # [N, 64]
    h = concat([x, agg, u[v_indices]], axis=1)                         # [N, 256]
    out = relu(h @ W1 + b1) @ W2 + b2                                  # [N, 128]

Strategy (8 NeuronCores, SPMD, no collectives):
  - Nodes are assigned to (core, window-of-32) slots by a degree-balanced
    snake deal: nodes sorted by degree are dealt round-robin (alternating
    direction) across all 3200 (core, window) bins, so every bin holds ~1/3200
    of all edges and almost every window needs exactly 4 zero-padded 128-edge
    tiles (the padding-free minimum is 3.9). Output is un-permuted on host.
  - edge_attr is sent as bf16 (hi half only; ~0.1% rel err, well under the
    2e-2 gate) => half the dominant HBM traffic. Everything else is bf16 too
    except biases; the output returns as bf16 and is widened on host.
  - DMA granularity: one ea/x/ug/out DMA per 8-block supergroup (1024 nodes).
    The HWDGE descriptor-generation path costs ~630ns per DMA instruction,
    serialized, so few fat DMAs matter as much as few bytes.
  - segment_sum on device: one-hot P[e, (m, t)] = (idx[e, t] == m) built for
    ALL tiles of a 128-node block in ONE DVE tensor_tensor. Layout [128, 32,
    T] with a materialized repeated-iota constant keeps every operand's last
    dim stride-1 so the DVE 2x 16-bit mode applies. Then TensorE matmuls
    aggT += ea.T @ P[:, :, t] per 32-node window, accumulated in PSUM; one
    PSUM tile and one ACT copy per 4-block MLP group.
  - MLP feature-major in bf16. agg (64 rows) and u-gather (64 rows) share one
    128-partition tile so h@W1 is 2 matmuls per output half instead of 3.
    ReLU is split between ACT (mh=0) and DVE (mh=1) to balance engines.
"""

import sys

sys.path.insert(0, "/opt/trn_rl_repo")

import numpy as np
import ml_dtypes

import concourse.bass as bass
import concourse.mybir as mybir
from concourse import bacc, tile
from concourse.bass_utils import run_bass_kernel_spmd

bf16 = ml_dtypes.bfloat16

D_X, D_E, D_U = 128, 64, 64
D_HID, D_OUT = 256, 128
NB = 128   # nodes per block
WSZ = 32   # nodes per one-hot window
WIN = NB // WSZ
SGB = 8    # blocks per DMA supergroup

FULL_CFG = dict(
    n_cores=8, n_nodes=100000, blocks=100, group=4
)  # 12800 node slots/core

_cache = {}


def _build_nc(Tb, blocks, npad, group, n_cores=8, reps=1, opts=None):
    """Build the SPMD Bass program. Tb = per-window edge tile counts
    (shared across cores; windows are 32 nodes, WIN windows per block).

    reps > 1 wraps the computation in a hardware For_i loop — used only
    for timing (per-iter time = delta(wall)/delta(reps), cancelling the
    host dispatch overhead)."""
    opts = dict(opts or {})
    ea_bufs = opts.get("ea_bufs", 3)
    p_bufs = opts.get("p_bufs", 6)
    Tb = list(Tb)
    offs = [0]
    for t in Tb:
        offs.append(offs[-1] + t)
    TT = offs[-1]
    max_blk_tiles = max(
        sum(Tb[b * WIN : (b + 1) * WIN]) for b in range(blocks)
    )
    sgs = [
        (s, min(s + SGB, blocks)) for s in range(0, blocks, SGB)
    ]
    max_sg_tiles = max(
        sum(Tb[s * WIN : e * WIN]) for s, e in sgs
    )
    nc = bacc.Bacc(
        "TRN2", target_bir_lowering=False, debug=False, num_devices=n_cores
    )
    f32, b16 = mybir.dt.float32, mybir.dt.bfloat16

    GW = group * NB       # nodes per MLP group
    SGW = SGB * NB        # nodes per supergroup
    gpsg = SGB // group   # MLP groups per supergroup

    # partition-major layouts; [K, mh, M] for weights
    ea_in = nc.declare_dram_parameter("ea", [128, TT * 64], b16, isOutput=False)
    idx_in = nc.declare_dram_parameter("idx", [128, TT], b16, isOutput=False)
    iota_in = nc.declare_dram_parameter("iota", [128, WSZ], b16, isOutput=False)
    xT_in = nc.declare_dram_parameter("xT", [128, npad], b16, isOutput=False)
    ugT_in = nc.declare_dram_parameter("ugT", [64, npad], b16, isOutput=False)
    w1x_in = nc.declare_dram_parameter("w1x", [128, 2, 128], b16, isOutput=False)
    w1au_in = nc.declare_dram_parameter("w1au", [128, 2, 128], b16, isOutput=False)
    w2_in = nc.declare_dram_parameter("w2", [128, 2, 128], b16, isOutput=False)
    b1_in = nc.declare_dram_parameter("b1", [128, 2], f32, isOutput=False)
    b2_in = nc.declare_dram_parameter("b2", [128, 1], f32, isOutput=False)
    outT = nc.declare_dram_parameter("outT", [128, npad], b16, isOutput=True)

    with tile.TileContext(nc) as tc:
        with (
            tc.tile_pool(name="const", bufs=1) as cpool,
            tc.tile_pool(name="x", bufs=3) as xpool,
            tc.tile_pool(name="ea", bufs=ea_bufs) as eapool,
            tc.tile_pool(name="p", bufs=p_bufs) as ppool,
            tc.tile_pool(name="aug", bufs=3) as augpool,
            tc.tile_pool(name="h1", bufs=4) as h1pool,
            tc.tile_pool(name="outs", bufs=3) as opool,
            tc.tile_pool(name="ps_agg", bufs=3, space="PSUM") as agg_ps_pool,
            tc.tile_pool(name="ps_o1", bufs=2, space="PSUM") as o1_ps_pool,
            tc.tile_pool(name="ps_o2", bufs=2, space="PSUM") as o2_ps_pool,
        ):
          def _emit_body():
              # ---- constants ----
              idx_t = cpool.tile([128, TT], b16, tag="idx")
              nc.sync.dma_start(idx_t[:], idx_in[:])
              iota_t = cpool.tile([128, WSZ], b16, tag="iota")
              nc.sync.dma_start(iota_t[:], iota_in[:])
              w1x_t = cpool.tile([128, 2, 128], b16, tag="w1x")
              nc.sync.dma_start(w1x_t[:], w1x_in[:])
              w1au_t = cpool.tile([128, 2, 128], b16, tag="w1au")
              nc.sync.dma_start(w1au_t[:], w1au_in[:])
              w2_t = cpool.tile([128, 2, 128], b16, tag="w2")
              nc.sync.dma_start(w2_t[:], w2_in[:])
              b1_t = cpool.tile([128, 2], f32, tag="b1")
              nc.sync.dma_start(b1_t[:], b1_in[:])
              b2_t = cpool.tile([128, 1], f32, tag="b2")
              nc.sync.dma_start(b2_t[:], b2_in[:])

              for sg, (bs, be) in enumerate(sgs):
                  nsg = (be - bs) * NB
                  s = bs * NB
                  o_sg = offs[bs * WIN]
                  Tsg = offs[be * WIN] - o_sg
                  # ---- supergroup DMAs ----
                  ea_t = eapool.tile(
                      [128, max_sg_tiles * 64], b16, tag="ea", name=f"ea{sg}"
                  )
                  nc.sync.dma_start(
                      ea_t[:, : Tsg * 64],
                      ea_in[:, o_sg * 64 : (o_sg + Tsg) * 64],
                  )
                  x_t = xpool.tile([128, SGW], b16, tag="x")
                  nc.sync.dma_start(x_t[:, :nsg], xT_in[:, s : s + nsg])
                  aug = augpool.tile([128, SGW], b16, tag="aug")
                  nc.scalar.dma_start(
                      aug[64:128, :nsg], ugT_in[:, s : s + nsg]
                  )
                  out_t = opool.tile([128, SGW], b16, tag="outs")

                  for g in range(gpsg):
                      gb = bs + g * group
                      if gb >= be:
                          break
                      gw = (min(gb + group, be) - gb) * NB
                      go = g * group * NB    # offset within supergroup
                      agg_ps = agg_ps_pool.tile([64, GW], f32, tag="agg")
                      for bi in range(group):
                          b = gb + bi
                          if b >= be:
                              break
                          Tws = Tb[b * WIN : (b + 1) * WIN]
                          o_b = offs[b * WIN] - o_sg   # tile offset in ea_t
                          Tblk = sum(Tws)
                          # one-hot for all tiles of the block in ONE DVE op
                          p_t = ppool.tile(
                              [128, max_blk_tiles, WSZ], b16, tag="p"
                          )
                          nc.vector.tensor_tensor(
                              out=p_t[:, 0:Tblk, :],
                              in0=idx_t[:, o_sg + o_b : o_sg + o_b + Tblk]
                              .unsqueeze(2)
                              .broadcast_to([128, Tblk, WSZ]),
                              in1=iota_t[:, 0:WSZ]
                              .unsqueeze(1)
                              .broadcast_to([128, Tblk, WSZ]),
                              op=mybir.AluOpType.is_equal,
                          )
                          # segment-sum via matmul per tile, acc per window
                          ti = 0
                          for w in range(WIN):
                              co = bi * NB + w * WSZ
                              for t in range(Tws[w]):
                                  nc.tensor.matmul(
                                      agg_ps[:, co : co + WSZ],
                                      ea_t[:, (o_b + ti) * 64
                                           : (o_b + ti + 1) * 64],
                                      p_t[:, ti, :],
                                      start=(t == 0),
                                      stop=(t == Tws[w] - 1),
                                  )
                                  ti += 1
                      # stage aggT into the combined [agg; ug] tile
                      nc.scalar.activation(
                          out=aug[0:64, go : go + gw],
                          in_=agg_ps[:, :gw],
                          func=mybir.ActivationFunctionType.Copy,
                      )
                      # ---- MLP for this group ----
                      h1_list = []
                      for mh in range(2):
                          o1 = o1_ps_pool.tile([128, GW], f32, tag="o1")
                          nc.tensor.matmul(
                              o1[:, :gw], w1x_t[:, mh, :],
                              x_t[:, go : go + gw],
                              start=True, stop=False,
                          )
                          nc.tensor.matmul(
                              o1[:, :gw], w1au_t[:, mh, :],
                              aug[:, go : go + gw],
                              start=False, stop=True,
                          )
                          h1 = h1pool.tile([128, GW], b16, tag="h1")
                          if mh == 0:
                              nc.scalar.activation(
                                  out=h1[:, :gw], in_=o1[:, :gw],
                                  func=mybir.ActivationFunctionType.Relu,
                                  bias=b1_t[:, 0:1],
                              )
                          else:
                              nc.vector.tensor_scalar(
                                  out=h1[:, :gw], in0=o1[:, :gw],
                                  scalar1=b1_t[:, 1:2], scalar2=0.0,
                                  op0=mybir.AluOpType.add,
                                  op1=mybir.AluOpType.max,
                              )
                          h1_list.append(h1)
                      o2 = o2_ps_pool.tile([128, GW], f32, tag="o2")
                      for kh in range(2):
                          nc.tensor.matmul(
                              o2[:, :gw], w2_t[:, kh, :],
                              h1_list[kh][:, :gw],
                              start=(kh == 0), stop=(kh == 1),
                          )
                      nc.scalar.activation(
                          out=out_t[:, go : go + gw], in_=o2[:, :gw],
                          func=mybir.ActivationFunctionType.Identity,
                          bias=b2_t[:],
                      )
                  nc.scalar.dma_start(
                      outT[:, s : s + nsg], out_t[:, :nsg]
                  )

          if reps == 1:
              _emit_body()
          else:
              with tc.For_i(0, reps, 1):
                  _emit_body()

    nc.compile()
    return nc


def _pack_inputs(x, edge_index, edge_attr, u, v_indices, W1, b1, W2, b2, cfg):
    """Host-side sharding: degree-balanced node permutation + edge packing."""
    n_cores, blocks = cfg["n_cores"], cfg["blocks"]
    n_nodes = cfg["n_nodes"]
    npad = blocks * NB
    nwin = npad // WSZ           # windows per core
    nbins = n_cores * nwin       # (core, window) bins
    nslots = nbins * WSZ
    row = np.asarray(edge_index[0], dtype=np.int64)
    ea = np.asarray(edge_attr, dtype=np.float32)
    x = np.asarray(x, dtype=np.float32)
    u = np.asarray(u, dtype=np.float32)
    v_indices = np.asarray(v_indices, dtype=np.int64)
    W1 = np.asarray(W1, dtype=np.float32)
    W2 = np.asarray(W2, dtype=np.float32)
    b1 = np.asarray(b1, dtype=np.float32)
    b2 = np.asarray(b2, dtype=np.float32)
    d_e = ea.shape[1]

    # ---- snake-deal nodes (sorted by degree desc) across bins ----
    deg = np.bincount(row, minlength=n_nodes)
    order = np.argsort(-deg, kind="stable")          # high degree first
    node_core = np.empty(n_nodes, np.int32)
    node_win = np.empty(n_nodes, np.int32)
    node_off = np.empty(n_nodes, np.int32)
    pos = np.arange(nslots)
    rounds, cols = pos // nbins, pos % nbins
    bins = np.where(rounds % 2 == 0, cols, nbins - 1 - cols)
    rb, bb = rounds[:n_nodes], bins[:n_nodes]
    node_core[order] = (bb // nwin).astype(np.int32)
    node_win[order] = (bb % nwin).astype(np.int32)
    node_off[order] = rb.astype(np.int32)
    node_plocal = node_win * WSZ + node_off          # slot within core

    # ---- edge buckets ----
    ec = node_core[row]
    ew = node_win[row]
    em = node_off[row]
    key = ec.astype(np.int64) * nwin + ew
    cnt = np.bincount(key, minlength=nbins).reshape(n_cores, nwin)
    Tb = np.maximum(1, (cnt.max(axis=0) + 127) // 128).astype(int)  # [nwin]
    offs = np.concatenate([[0], np.cumsum(Tb)])
    TT = int(offs[-1])

    order_e = np.argsort(key, kind="stable")
    key_s = key[order_e]
    cnt_flat = np.bincount(key_s, minlength=nbins)
    starts_flat = np.concatenate([[0], np.cumsum(cnt_flat)])[:-1]
    rank = np.arange(len(key_s)) - starts_flat[key_s]
    ew_s = ew[order_e]
    slot = offs[ew_s] * 128 + rank                   # within-core slot
    ec_s = ec[order_e]
    em_s = em[order_e].astype(np.float32)
    ea_hi = ea[order_e].astype(bf16)

    ea_pack = np.empty((n_cores, 128, TT * 64), dtype=bf16)
    idx_pack = np.empty((n_cores, 128, TT), dtype=bf16)
    for c in range(n_cores):
        m = ec_s == c
        coreslots = np.zeros((TT * 128, d_e), dtype=bf16)
        coreslots[slot[m]] = ea_hi[m]
        ea_pack[c] = (
            coreslots.reshape(TT, 128, d_e).transpose(1, 0, 2).reshape(128, -1)
        )
        ivals = np.zeros(TT * 128, dtype=np.float32)
        ivals[slot[m]] = em_s[m]
        idx_pack[c] = ivals.reshape(TT, 128).T.astype(bf16)

    iota = np.broadcast_to(
        np.arange(WSZ, dtype=np.float32), (128, WSZ)
    ).astype(bf16)
    uT = u.T  # [d_u, n_graphs]

    w1x = np.ascontiguousarray(W1[:D_X].reshape(D_X, 2, 128)).astype(bf16)
    w1au = np.ascontiguousarray(W1[D_X:].reshape(128, 2, 128)).astype(bf16)
    w2 = np.ascontiguousarray(
        W2.reshape(2, 128, D_OUT).transpose(1, 0, 2)
    ).astype(bf16)
    b1p = np.ascontiguousarray(b1.reshape(2, 128).T)
    b2p = np.ascontiguousarray(b2.reshape(128, 1))

    in_maps = []
    for c in range(n_cores):
        sel = node_core == c
        pl = node_plocal[sel]
        xT = np.zeros((D_X, npad), dtype=bf16)
        xT[:, pl] = x[sel].T.astype(bf16)
        ugT = np.zeros((D_U, npad), dtype=bf16)
        ugT[:, pl] = uT[:, v_indices[sel]].astype(bf16)
        in_maps.append({
            "ea": ea_pack[c],
            "idx": idx_pack[c],
            "iota": iota,
            "xT": xT,
            "ugT": ugT,
            "w1x": w1x,
            "w1au": w1au,
            "w2": w2,
            "b1": b1p,
            "b2": b2p,
        })
    unperm = (node_core, node_plocal)
    return in_maps, tuple(int(t) for t in Tb), unperm


def _unpack_output(res_per_core, unperm, cfg):
    node_core, node_plocal = unperm
    n_nodes = cfg["n_nodes"]
    out = np.empty((n_nodes, D_OUT), dtype=np.float32)
    for c in range(cfg["n_cores"]):
        sel = node_core == c
        out[sel] = np.asarray(res_per_core[c]).astype(np.float32).T[
            node_plocal[sel]
        ]
    return out


def _run(inputs, cfg, reps=1):
    in_maps, T, unperm = _pack_inputs(
        inputs["x"], inputs["edge_index"], inputs["edge_attr"], inputs["u"],
        inputs["v_indices"], inputs["W1"], inputs["b1"], inputs["W2"],
        inputs["b2"], cfg,
    )
    key = (T, cfg["blocks"], cfg["group"], reps)
    if key not in _cache:
        _cache[key] = _build_nc(
            T, cfg["blocks"], cfg["blocks"] * NB, cfg["group"],
            n_cores=cfg["n_cores"], reps=reps,
        )
    nc = _cache[key]
    res = run_bass_kernel_spmd(nc, in_maps, list(range(cfg["n_cores"])))
    return _unpack_output(
        [res.results[c]["outT"] for c in range(cfg["n_cores"])], unperm, cfg
    )


def kernel(x, edge_index, edge_attr, u, v_indices, W1, b1, W2, b2):
    inputs = dict(x=x, edge_index=edge_index, edge_attr=edge_attr, u=u,
                  v_indices=v_indices, W1=W1, b1=b1, W2=W2, b2=b2)
    return _run(inputs, FULL_CFG)


# revision 10
# speedup vs baseline: 1.4001x; 1.0411x over previous
"""Trainium2 Bass kernel for nn_NodeModel (GNN message passing).

reference:
    agg = segment_sum(edge_attr, edge_index[0], num_segments=100000)   # [N, 64]
    h = concat([x, agg, u[v_indices]], axis=1)                         # [N, 256]
    out = relu(h @ W1 + b1) @ W2 + b2                                  # [N, 128]

Strategy (8 NeuronCores, SPMD, no collectives):
  - Nodes are assigned to (core, window-of-32) slots by a degree-balanced
    snake deal: nodes sorted by degree are dealt round-robin (alternating
    direction) across all 3200 (core, window) bins, so every bin holds ~1/3200
    of all edges and almost every window needs exactly 4 zero-padded 128-edge
    tiles (the padding-free minimum is 3.9). Output is un-permuted on host.
  - edge_attr is sent as bf16 (hi half only; ~0.1% rel err, well under the
    2e-2 gate) => half the dominant HBM traffic. Everything else is bf16 too
    except biases; the output returns as bf16 and is widened on host.
  - DMA granularity: one ea/x/ug/out DMA per 8-block supergroup (1024 nodes).
    The HWDGE descriptor-generation path costs ~630ns per DMA instruction,
    serialized, so few fat DMAs matter as much as few bytes.
  - segment_sum on device: one-hot P[e, (m, t)] = (idx[e, t] == m) built for
    ALL tiles of a 128-node block in ONE DVE tensor_tensor. Layout [128, 32,
    T] with a materialized repeated-iota constant keeps every operand's last
    dim stride-1 so the DVE 2x 16-bit mode applies. Then TensorE matmuls
    aggT += ea.T @ P[:, :, t] per 32-node window, accumulated in PSUM; one
    PSUM tile and one ACT copy per 4-block MLP group.
  - MLP feature-major in bf16. agg (64 rows) and u-gather (64 rows) share one
    128-partition tile so h@W1 is 2 matmuls per output half instead of 3.
    ReLU is split between ACT (mh=0) and DVE (mh=1) to balance engines.
"""

import sys

sys.path.insert(0, "/opt/trn_rl_repo")

import numpy as np
import ml_dtypes

import concourse.bass as bass
import concourse.mybir as mybir
from concourse import bacc, tile
from concourse.bass_utils import run_bass_kernel_spmd

bf16 = ml_dtypes.bfloat16

D_X, D_E, D_U = 128, 64, 64
D_HID, D_OUT = 256, 128
NB = 128   # nodes per block
WSZ = 32   # nodes per one-hot window
WIN = NB // WSZ
SGB = 8    # blocks per DMA supergroup

FULL_CFG = dict(
    n_cores=8, n_nodes=100000, blocks=100, group=4
)  # 12800 node slots/core

_cache = {}


def _build_nc(Tb, blocks, npad, group, n_cores=8, reps=1, opts=None):
    """Build the SPMD Bass program. Tb = per-window edge tile counts
    (shared across cores; windows are 32 nodes, WIN windows per block).

    reps > 1 wraps the computation in a hardware For_i loop — used only
    for timing (per-iter time = delta(wall)/delta(reps), cancelling the
    host dispatch overhead)."""
    opts = dict(opts or {})
    dma_fine = opts.get("dma_fine", False)   # per-block ea, per-group x/ug/out
    agg_per_block = opts.get("agg_per_block", False)
    out_f32 = opts.get("out_f32", False)
    probe = opts.get("probe")                # None | "pe" | "dma" | "dve"
    ea_bufs = opts.get("ea_bufs", 6 if dma_fine else 3)
    p_bufs = opts.get("p_bufs", 6)
    Tb = list(Tb)
    offs = [0]
    for t in Tb:
        offs.append(offs[-1] + t)
    TT = offs[-1]
    max_blk_tiles = max(
        sum(Tb[b * WIN : (b + 1) * WIN]) for b in range(blocks)
    )
    sgs = [
        (s, min(s + SGB, blocks)) for s in range(0, blocks, SGB)
    ]
    max_sg_tiles = max(
        sum(Tb[s * WIN : e * WIN]) for s, e in sgs
    )
    nc = bacc.Bacc(
        "TRN2", target_bir_lowering=False, debug=False, num_devices=n_cores
    )
    f32, b16 = mybir.dt.float32, mybir.dt.bfloat16

    GW = group * NB       # nodes per MLP group
    SGW = SGB * NB        # nodes per supergroup
    gpsg = SGB // group   # MLP groups per supergroup

    # partition-major layouts; [K, mh, M] for weights
    ea_in = nc.declare_dram_parameter("ea", [128, TT * 64], b16, isOutput=False)
    idx_in = nc.declare_dram_parameter("idx", [128, TT], b16, isOutput=False)
    iota_in = nc.declare_dram_parameter("iota", [128, WSZ], b16, isOutput=False)
    xT_in = nc.declare_dram_parameter("xT", [128, npad], b16, isOutput=False)
    ugT_in = nc.declare_dram_parameter("ugT", [64, npad], b16, isOutput=False)
    w1x_in = nc.declare_dram_parameter("w1x", [128, 2, 128], b16, isOutput=False)
    w1au_in = nc.declare_dram_parameter("w1au", [128, 2, 128], b16, isOutput=False)
    w2_in = nc.declare_dram_parameter("w2", [128, 2, 128], b16, isOutput=False)
    b1_in = nc.declare_dram_parameter("b1", [128, 2], f32, isOutput=False)
    b2_in = nc.declare_dram_parameter("b2", [128, 1], f32, isOutput=False)
    outT = nc.declare_dram_parameter(
        "outT", [128, npad], f32 if out_f32 else b16, isOutput=True
    )

    with tile.TileContext(nc) as tc:
        with (
            tc.tile_pool(name="const", bufs=1) as cpool,
            tc.tile_pool(name="x", bufs=3) as xpool,
            tc.tile_pool(name="ea", bufs=ea_bufs) as eapool,
            tc.tile_pool(name="p", bufs=p_bufs) as ppool,
            tc.tile_pool(name="aug", bufs=3) as augpool,
            tc.tile_pool(name="h1", bufs=4) as h1pool,
            tc.tile_pool(name="outs", bufs=3) as opool,
            tc.tile_pool(name="ps_agg", bufs=3, space="PSUM") as agg_ps_pool,
            tc.tile_pool(name="ps_o1", bufs=2, space="PSUM") as o1_ps_pool,
            tc.tile_pool(name="ps_o2", bufs=2, space="PSUM") as o2_ps_pool,
        ):
          def _emit_body():
              # ---- constants ----
              idx_t = cpool.tile([128, TT], b16, tag="idx")
              nc.sync.dma_start(idx_t[:], idx_in[:])
              iota_t = cpool.tile([128, WSZ], b16, tag="iota")
              nc.sync.dma_start(iota_t[:], iota_in[:])
              w1x_t = cpool.tile([128, 2, 128], b16, tag="w1x")
              nc.sync.dma_start(w1x_t[:], w1x_in[:])
              w1au_t = cpool.tile([128, 2, 128], b16, tag="w1au")
              nc.sync.dma_start(w1au_t[:], w1au_in[:])
              w2_t = cpool.tile([128, 2, 128], b16, tag="w2")
              nc.sync.dma_start(w2_t[:], w2_in[:])
              b1_t = cpool.tile([128, 2], f32, tag="b1")
              nc.sync.dma_start(b1_t[:], b1_in[:])
              b2_t = cpool.tile([128, 1], f32, tag="b2")
              nc.sync.dma_start(b2_t[:], b2_in[:])

              if probe == "pe":
                  # pure PE throughput: full matmul schedule on const tiles
                  ea_c = eapool.tile([128, max_blk_tiles * 64], b16, tag="ea")
                  nc.sync.dma_start(
                      ea_c[:], ea_in[:, : max_blk_tiles * 64]
                  )
                  p_c = ppool.tile([128, max_blk_tiles, WSZ], b16, tag="p")
                  nc.vector.tensor_tensor(
                      out=p_c[:],
                      in0=idx_t[:, 0:max_blk_tiles].unsqueeze(2)
                      .broadcast_to([128, max_blk_tiles, WSZ]),
                      in1=iota_t[:, 0:WSZ].unsqueeze(1)
                      .broadcast_to([128, max_blk_tiles, WSZ]),
                      op=mybir.AluOpType.is_equal,
                  )
                  x_c = xpool.tile([128, GW], b16, tag="x")
                  nc.sync.dma_start(x_c[:], xT_in[:, :GW])
                  h_c = h1pool.tile([128, GW], b16, tag="h1")
                  nc.scalar.activation(
                      out=h_c[:], in_=x_c[:],
                      func=mybir.ActivationFunctionType.Copy,
                  )
                  for b in range(blocks):
                      Tws = Tb[b * WIN : (b + 1) * WIN]
                      agg_ps = agg_ps_pool.tile([64, NB], f32, tag="agg")
                      ti = 0
                      for w in range(WIN):
                          for t in range(Tws[w]):
                              nc.tensor.matmul(
                                  agg_ps[:, w * WSZ : (w + 1) * WSZ],
                                  ea_c[:, ti * 64 : (ti + 1) * 64],
                                  p_c[:, ti, :],
                                  start=(t == 0), stop=(t == Tws[w] - 1),
                              )
                              ti += 1
                      if b % group == group - 1:
                          for mh in range(2):
                              o1 = o1_ps_pool.tile([128, GW], f32, tag="o1")
                              nc.tensor.matmul(
                                  o1[:], w1x_t[:, mh, :], x_c[:],
                                  start=True, stop=False,
                              )
                              nc.tensor.matmul(
                                  o1[:], w1au_t[:, mh, :], x_c[:],
                                  start=False, stop=True,
                              )
                          o2 = o2_ps_pool.tile([128, GW], f32, tag="o2")
                          for kh in range(2):
                              nc.tensor.matmul(
                                  o2[:], w2_t[:, kh, :], h_c[:],
                                  start=(kh == 0), stop=(kh == 1),
                              )
                  return
              if probe == "dma":
                  # pure DMA floor: all input streams, no compute
                  for sg, (bs, be) in enumerate(sgs):
                      nsg = (be - bs) * NB
                      s = bs * NB
                      o_sg = offs[bs * WIN]
                      Tsg = offs[be * WIN] - o_sg
                      ea_sg = eapool.tile(
                          [128, max_sg_tiles * 64], b16, tag="ea"
                      )
                      nc.sync.dma_start(
                          ea_sg[:, : Tsg * 64],
                          ea_in[:, o_sg * 64 : (o_sg + Tsg) * 64],
                      )
                      x_t = xpool.tile([128, SGW], b16, tag="x")
                      nc.sync.dma_start(
                          x_t[:, :nsg], xT_in[:, s : s + nsg]
                      )
                      aug = augpool.tile([128, SGW], b16, tag="aug")
                      nc.scalar.dma_start(
                          aug[64:128, :nsg], ugT_in[:, s : s + nsg]
                      )
                  return
              out_dt = f32 if out_f32 else b16
              for sg, (bs, be) in enumerate(sgs):
                  nsg = (be - bs) * NB
                  s = bs * NB
                  o_sg = offs[bs * WIN]
                  Tsg = offs[be * WIN] - o_sg
                  # ---- supergroup DMAs ----
                  if not dma_fine:
                      ea_sg = eapool.tile(
                          [128, max_sg_tiles * 64], b16, tag="ea",
                          name=f"ea{sg}"
                      )
                      nc.sync.dma_start(
                          ea_sg[:, : Tsg * 64],
                          ea_in[:, o_sg * 64 : (o_sg + Tsg) * 64],
                      )
                      x_t = xpool.tile([128, SGW], b16, tag="x")
                      nc.sync.dma_start(
                          x_t[:, :nsg], xT_in[:, s : s + nsg]
                      )
                      aug = augpool.tile([128, SGW], b16, tag="aug")
                      nc.scalar.dma_start(
                          aug[64:128, :nsg], ugT_in[:, s : s + nsg]
                      )
                      out_t = opool.tile([128, SGW], out_dt, tag="outs")

                  for g in range(gpsg):
                      gb = bs + g * group
                      if gb >= be:
                          break
                      gw = (min(gb + group, be) - gb) * NB
                      go = g * group * NB    # offset within supergroup
                      gs = s + go            # offset within core
                      if dma_fine:
                          x_t = xpool.tile([128, GW], b16, tag="x")
                          nc.sync.dma_start(
                              x_t[:, :gw], xT_in[:, gs : gs + gw]
                          )
                          aug = augpool.tile([128, GW], b16, tag="aug")
                          nc.scalar.dma_start(
                              aug[64:128, :gw], ugT_in[:, gs : gs + gw]
                          )
                          out_t = opool.tile([128, GW], out_dt, tag="outs")
                          go = 0
                      if not agg_per_block:
                          agg_ps = agg_ps_pool.tile([64, GW], f32, tag="agg")
                      for bi in range(group):
                          b = gb + bi
                          if b >= be:
                              break
                          Tws = Tb[b * WIN : (b + 1) * WIN]
                          Tblk = sum(Tws)
                          ob_g = offs[b * WIN]         # global tile offset
                          if dma_fine:
                              ea_t = eapool.tile(
                                  [128, max_blk_tiles * 64], b16, tag="ea",
                                  name=f"ea{b}"
                              )
                              nc.sync.dma_start(
                                  ea_t[:, : Tblk * 64],
                                  ea_in[:, ob_g * 64 : (ob_g + Tblk) * 64],
                              )
                              o_b = 0
                          else:
                              ea_t = ea_sg
                              o_b = ob_g - o_sg        # tile offset in ea_sg
                          if agg_per_block:
                              agg_ps = agg_ps_pool.tile(
                                  [64, NB], f32, tag="agg"
                              )
                              co0 = 0
                          else:
                              co0 = bi * NB
                          # one-hot for all tiles of the block in ONE DVE op
                          p_t = ppool.tile(
                              [128, max_blk_tiles, WSZ], b16, tag="p"
                          )
                          nc.vector.tensor_tensor(
                              out=p_t[:, 0:Tblk, :],
                              in0=idx_t[:, ob_g : ob_g + Tblk]
                              .unsqueeze(2)
                              .broadcast_to([128, Tblk, WSZ]),
                              in1=iota_t[:, 0:WSZ]
                              .unsqueeze(1)
                              .broadcast_to([128, Tblk, WSZ]),
                              op=mybir.AluOpType.is_equal,
                          )
                          # segment-sum via matmul per tile, acc per window
                          ti = 0
                          for w in range(WIN):
                              co = co0 + w * WSZ
                              for t in range(Tws[w]):
                                  nc.tensor.matmul(
                                      agg_ps[:, co : co + WSZ],
                                      ea_t[:, (o_b + ti) * 64
                                           : (o_b + ti + 1) * 64],
                                      p_t[:, ti, :],
                                      start=(t == 0),
                                      stop=(t == Tws[w] - 1),
                                  )
                                  ti += 1
                          if agg_per_block:
                              nc.scalar.activation(
                                  out=aug[0:64,
                                          go + bi * NB : go + (bi + 1) * NB],
                                  in_=agg_ps[:],
                                  func=mybir.ActivationFunctionType.Copy,
                              )
                      if not agg_per_block:
                          # stage aggT into the combined [agg; ug] tile
                          nc.scalar.activation(
                              out=aug[0:64, go : go + gw],
                              in_=agg_ps[:, :gw],
                              func=mybir.ActivationFunctionType.Copy,
                          )
                      # ---- MLP for this group ----
                      h1_list = []
                      for mh in range(2):
                          o1 = o1_ps_pool.tile([128, GW], f32, tag="o1")
                          nc.tensor.matmul(
                              o1[:, :gw], w1x_t[:, mh, :],
                              x_t[:, go : go + gw],
                              start=True, stop=False,
                          )
                          nc.tensor.matmul(
                              o1[:, :gw], w1au_t[:, mh, :],
                              aug[:, go : go + gw],
                              start=False, stop=True,
                          )
                          h1 = h1pool.tile([128, GW], b16, tag="h1")
                          if mh == 0:
                              nc.scalar.activation(
                                  out=h1[:, :gw], in_=o1[:, :gw],
                                  func=mybir.ActivationFunctionType.Relu,
                                  bias=b1_t[:, 0:1],
                              )
                          else:
                              nc.vector.tensor_scalar(
                                  out=h1[:, :gw], in0=o1[:, :gw],
                                  scalar1=b1_t[:, 1:2], scalar2=0.0,
                                  op0=mybir.AluOpType.add,
                                  op1=mybir.AluOpType.max,
                              )
                          h1_list.append(h1)
                      o2 = o2_ps_pool.tile([128, GW], f32, tag="o2")
                      for kh in range(2):
                          nc.tensor.matmul(
                              o2[:, :gw], w2_t[:, kh, :],
                              h1_list[kh][:, :gw],
                              start=(kh == 0), stop=(kh == 1),
                          )
                      nc.scalar.activation(
                          out=out_t[:, go : go + gw], in_=o2[:, :gw],
                          func=mybir.ActivationFunctionType.Identity,
                          bias=b2_t[:],
                      )
                      if dma_fine:
                          nc.scalar.dma_start(
                              outT[:, gs : gs + gw], out_t[:, :gw]
                          )
                  if not dma_fine:
                      nc.scalar.dma_start(
                          outT[:, s : s + nsg], out_t[:, :nsg]
                      )

          if reps == 1:
              _emit_body()
          else:
              with tc.For_i(0, reps, 1):
                  _emit_body()

    nc.compile()
    return nc


def _pack_inputs(x, edge_index, edge_attr, u, v_indices, W1, b1, W2, b2, cfg):
    """Host-side sharding: degree-balanced node permutation + edge packing."""
    n_cores, blocks = cfg["n_cores"], cfg["blocks"]
    n_nodes = cfg["n_nodes"]
    npad = blocks * NB
    nwin = npad // WSZ           # windows per core
    nbins = n_cores * nwin       # (core, window) bins
    nslots = nbins * WSZ
    row = np.asarray(edge_index[0], dtype=np.int64)
    ea = np.asarray(edge_attr, dtype=np.float32)
    x = np.asarray(x, dtype=np.float32)
    u = np.asarray(u, dtype=np.float32)
    v_indices = np.asarray(v_indices, dtype=np.int64)
    W1 = np.asarray(W1, dtype=np.float32)
    W2 = np.asarray(W2, dtype=np.float32)
    b1 = np.asarray(b1, dtype=np.float32)
    b2 = np.asarray(b2, dtype=np.float32)
    d_e = ea.shape[1]

    # ---- snake-deal nodes (sorted by degree desc) across bins ----
    deg = np.bincount(row, minlength=n_nodes)
    order = np.argsort(-deg, kind="stable")          # high degree first
    node_core = np.empty(n_nodes, np.int32)
    node_win = np.empty(n_nodes, np.int32)
    node_off = np.empty(n_nodes, np.int32)
    pos = np.arange(nslots)
    rounds, cols = pos // nbins, pos % nbins
    bins = np.where(rounds % 2 == 0, cols, nbins - 1 - cols)
    rb, bb = rounds[:n_nodes], bins[:n_nodes]
    node_core[order] = (bb // nwin).astype(np.int32)
    node_win[order] = (bb % nwin).astype(np.int32)
    node_off[order] = rb.astype(np.int32)
    node_plocal = node_win * WSZ + node_off          # slot within core

    # ---- edge buckets ----
    ec = node_core[row]
    ew = node_win[row]
    em = node_off[row]
    key = ec.astype(np.int64) * nwin + ew
    cnt = np.bincount(key, minlength=nbins).reshape(n_cores, nwin)
    Tb = np.maximum(1, (cnt.max(axis=0) + 127) // 128).astype(int)  # [nwin]
    offs = np.concatenate([[0], np.cumsum(Tb)])
    TT = int(offs[-1])

    order_e = np.argsort(key, kind="stable")
    key_s = key[order_e]
    cnt_flat = np.bincount(key_s, minlength=nbins)
    starts_flat = np.concatenate([[0], np.cumsum(cnt_flat)])[:-1]
    rank = np.arange(len(key_s)) - starts_flat[key_s]
    ew_s = ew[order_e]
    slot = offs[ew_s] * 128 + rank                   # within-core slot
    ec_s = ec[order_e]
    em_s = em[order_e].astype(np.float32)
    ea_hi = ea[order_e].astype(bf16)

    ea_pack = np.empty((n_cores, 128, TT * 64), dtype=bf16)
    idx_pack = np.empty((n_cores, 128, TT), dtype=bf16)
    for c in range(n_cores):
        m = ec_s == c
        coreslots = np.zeros((TT * 128, d_e), dtype=bf16)
        coreslots[slot[m]] = ea_hi[m]
        ea_pack[c] = (
            coreslots.reshape(TT, 128, d_e).transpose(1, 0, 2).reshape(128, -1)
        )
        ivals = np.zeros(TT * 128, dtype=np.float32)
        ivals[slot[m]] = em_s[m]
        idx_pack[c] = ivals.reshape(TT, 128).T.astype(bf16)

    iota = np.broadcast_to(
        np.arange(WSZ, dtype=np.float32), (128, WSZ)
    ).astype(bf16)
    uT = u.T  # [d_u, n_graphs]

    w1x = np.ascontiguousarray(W1[:D_X].reshape(D_X, 2, 128)).astype(bf16)
    w1au = np.ascontiguousarray(W1[D_X:].reshape(128, 2, 128)).astype(bf16)
    w2 = np.ascontiguousarray(
        W2.reshape(2, 128, D_OUT).transpose(1, 0, 2)
    ).astype(bf16)
    b1p = np.ascontiguousarray(b1.reshape(2, 128).T)
    b2p = np.ascontiguousarray(b2.reshape(128, 1))

    in_maps = []
    for c in range(n_cores):
        sel = node_core == c
        pl = node_plocal[sel]
        xT = np.zeros((D_X, npad), dtype=bf16)
        xT[:, pl] = x[sel].T.astype(bf16)
        ugT = np.zeros((D_U, npad), dtype=bf16)
        ugT[:, pl] = uT[:, v_indices[sel]].astype(bf16)
        in_maps.append({
            "ea": ea_pack[c],
            "idx": idx_pack[c],
            "iota": iota,
            "xT": xT,
            "ugT": ugT,
            "w1x": w1x,
            "w1au": w1au,
            "w2": w2,
            "b1": b1p,
            "b2": b2p,
        })
    unperm = (node_core, node_plocal)
    return in_maps, tuple(int(t) for t in Tb), unperm


def _unpack_output(res_per_core, unperm, cfg):
    node_core, node_plocal = unperm
    n_nodes = cfg["n_nodes"]
    out = np.empty((n_nodes, D_OUT), dtype=np.float32)
    for c in range(cfg["n_cores"]):
        sel = node_core == c
        out[sel] = np.asarray(res_per_core[c]).astype(np.float32).T[
            node_plocal[sel]
        ]
    return out


def _run(inputs, cfg, reps=1):
    in_maps, T, unperm = _pack_inputs(
        inputs["x"], inputs["edge_index"], inputs["edge_attr"], inputs["u"],
        inputs["v_indices"], inputs["W1"], inputs["b1"], inputs["W2"],
        inputs["b2"], cfg,
    )
    key = (T, cfg["blocks"], cfg["group"], reps)
    if key not in _cache:
        _cache[key] = _build_nc(
            T, cfg["blocks"], cfg["blocks"] * NB, cfg["group"],
            n_cores=cfg["n_cores"], reps=reps,
        )
    nc = _cache[key]
    res = run_bass_kernel_spmd(nc, in_maps, list(range(cfg["n_cores"])))
    return _unpack_output(
        [res.results[c]["outT"] for c in range(cfg["n_cores"])], unperm, cfg
    )


def kernel(x, edge_index, edge_attr, u, v_indices, W1, b1, W2, b2):
    inputs = dict(x=x, edge_index=edge_index, edge_attr=edge_attr, u=u,
                  v_indices=v_indices, W1=W1, b1=b1, W2=W2, b2=b2)
    return _run(inputs, FULL_CFG)
